# revision 1
# baseline (speedup 1.0000x reference)
"""Equivariant MLP (9 -> 49 -> 49 -> 9, tied weights) on 8 trn2 NeuronCores.

Data parallel over the batch (1048576 rows -> 131072/core).  Tied-weight
patterns are expanded to dense matrices on the host (tiny gathers).  The
device runs feature-major: the host hands each core x^T as a banded
[4, 18, 16384] array — band j holds batch-block pair (2j, 2j+1) stacked on
9+9 partitions — which the kernel DMAs into SBUF partitions {32j..32j+17}
(32-aligned so each pair can be a matmul operand).  Per pair: L1 matmul with
block-diagonal [18, 98] weights, fused bias+relu on ACT (PSUM->SBUF), L2
[98, 98] matmul, fused bias+relu on DVE, then L3 as four accumulating
matmuls with zero-padded [98, 72] weights packing all four pairs' y^T into
one [72, C] PSUM tile (one cheap evacuation).  Matmuls use float32r (fp32
bits, single-pass PE mode: 1 cycle/column vs 4 for strict fp32).
"""

import os
import sys

sys.path.insert(0, "/opt/trn_rl_repo")

import numpy as np

import concourse.bass as bass
import concourse.mybir as mybir
import concourse.tile as tile
from concourse.bass_utils import run_bass_kernel_spmd

f32 = mybir.dt.float32
f32r = mybir.dt.float32r

N_CORES = 8
BATCH = 1048576
BS = BATCH // N_CORES          # 131072 rows per core
NBLK = 8                       # batch blocks per core (4 pairs)
S = BS // NBLK                 # 16384 columns per block
C = 1024                       # columns per strip (DMA + evac width)
MM = 512                       # columns per matmul (PSUM bank limit)

last_exec_ns = None


def _split_multi_waits(nc):
    """Walrus in this container rejects instructions carrying more than one
    sync wait ("Too many sync wait commands", e.g. Drain and Ldweights
    encodings).  Re-park all but one wait of every instruction on same-engine
    NoOps inserted just before it."""
    n = 0
    for fn in nc.m.functions:
        for bb in fn.blocks:
            out = []
            for inst in bb.instructions:
                si = inst.sync_info
                waits = list(si.on_wait) if (si and si.on_wait) else []
                if len(waits) > 1:
                    si.on_wait = waits[-1:]
                    for w in waits[:-1]:
                        nop = mybir.InstNoOp(name=f"WSPLIT-{n}", ins=[], outs=[])
                        n += 1
                        nop.engine = inst.engine
                        nop.sync_info = mybir.SyncInfo(on_update=[], on_wait=[w])
                        out.append(nop)
                out.append(inst)
            bb.instructions = out


def _build_nc(mm_dtype=f32r, c=None, hp_bufs=3, hp2_bufs=6, yp_bufs=3,
              ps_bufs=2):
    nc = bass.Bass()
    xt = nc.dram_tensor("xt", [4, 18, S], mm_dtype, kind="ExternalInput")
    w1 = nc.dram_tensor("w1", [128, 98], mm_dtype, kind="ExternalInput")
    w2 = nc.dram_tensor("w2", [98, 98], mm_dtype, kind="ExternalInput")
    w3x = nc.dram_tensor("w3x", [98, 4, 72], mm_dtype, kind="ExternalInput")
    b1 = nc.dram_tensor("b1", [98, 1], f32, kind="ExternalInput")
    b2 = nc.dram_tensor("b2", [98, 1], f32, kind="ExternalInput")
    b3 = nc.dram_tensor("b3", [72, 1], f32, kind="ExternalInput")
    yt = nc.dram_tensor("yt", [72, S], f32, kind="ExternalOutput")

    relu = mybir.ActivationFunctionType.Relu
    ident = mybir.ActivationFunctionType.Identity
    add = mybir.AluOpType.add
    amax = mybir.AluOpType.max

    C = c or globals()["C"]
    with tile.TileContext(nc) as tc:
        with (
            tc.tile_pool(name="consts", bufs=1) as cp,
            tc.tile_pool(name="hid", bufs=hp_bufs) as hp,
            tc.tile_pool(name="hid2", bufs=hp2_bufs) as hp2,
            tc.tile_pool(name="out", bufs=yp_bufs) as yp,
            tc.tile_pool(name="psum", bufs=ps_bufs, space=bass.MemorySpace.PSUM) as pp,
        ):
            w1t = cp.tile([128, 98], mm_dtype)
            nc.sync.dma_start(w1t[:], w1[:])
            w2t = cp.tile([98, 98], mm_dtype)
            nc.sync.dma_start(w2t[:], w2[:])
            w3t = cp.tile([98, 4, 72], mm_dtype)
            nc.sync.dma_start(w3t[:], w3x[:])
            b1t = cp.tile([98, 1], f32)
            nc.sync.dma_start(b1t[:], b1[:])
            b2t = cp.tile([98, 1], f32)
            nc.sync.dma_start(b2t[:], b2[:])
            b3t = cp.tile([72, 1], f32)
            nc.sync.dma_start(b3t[:], b3[:])

            # Static double-buffered x tiles: memset once so the band gap
            # partitions (32j+18..32j+31) stay zero forever; the matching
            # weight rows are zero too, so any round-up reads contribute 0.
            xtiles = []
            for i in range(2):
                xtl = cp.tile([128, C], mm_dtype, tag=f"x{i}")
                nc.vector.memset(xtl[:].bitcast(f32), 0.0)
                xtiles.append(xtl)

            for s_i, c0 in enumerate(range(0, S, C)):
                xtile = xtiles[s_i % 2]
                for j in range(4):
                    nc.sync.dma_start(
                        xtile[32 * j : 32 * j + 18, :], xt[j, :, c0 : c0 + C]
                    )

                h2s = []
                for j in range(4):
                    p1 = pp.tile([98, C], f32, tag="ps1")
                    for m in range(0, C, MM):
                        kw = {"tile_position": (96, 0)} if j == 3 else {}
                        nc.tensor.matmul(
                            p1[:, m : m + MM],
                            w1t[32 * j : 32 * j + 18, :],
                            xtile[32 * j : 32 * j + 18, m : m + MM],
                            start=True,
                            stop=True,
                            **kw,
                        )
                    h1 = hp.tile([98, C], mm_dtype, tag="h1")
                    nc.scalar.activation(h1[:], p1[:], relu, bias=b1t[:, 0:1])

                    p2 = pp.tile([98, C], f32, tag="ps2")
                    for m in range(0, C, MM):
                        nc.tensor.matmul(
                            p2[:, m : m + MM],
                            w2t[:],
                            h1[:, m : m + MM],
                            start=True,
                            stop=True,
                        )
                    h2 = hp2.tile([98, C], mm_dtype, tag="h2")
                    nc.vector.tensor_scalar(
                        h2[:], p2[:], b2t[:, 0:1], 0.0, add, amax
                    )
                    h2s.append(h2)

                p3 = pp.tile([72, C], f32, tag="ps1")
                for m in range(0, C, MM):
                    for j in range(4):
                        nc.tensor.matmul(
                            p3[:, m : m + MM],
                            w3t[:, j, :],
                            h2s[j][:, m : m + MM],
                            start=(j == 0),
                            stop=(j == 3),
                        )
                ytile = yp.tile([72, C], f32, tag="y")
                nc.scalar.activation(
                    ytile[:, 0:MM], p3[:, 0:MM], ident, bias=b3t[:, 0:1]
                )
                nc.vector.tensor_scalar(
                    ytile[:, MM:C], p3[:, MM:C], b3t[:, 0:1], None, add
                )
                nc.sync.dma_start(yt[:, c0 : c0 + C], ytile[:])
    _split_multi_waits(nc)
    return nc


_nc_cache = {}


def _get_nc(mm_dtype):
    key = str(mm_dtype)
    if key not in _nc_cache:
        _nc_cache[key] = _build_nc(mm_dtype)
    return _nc_cache[key]


def _expand(pattern, params):
    pattern = np.asarray(pattern)
    params = np.asarray(params, np.float32)
    return np.where(pattern > 0, params[np.maximum(pattern - 1, 0)], 0.0).astype(
        np.float32
    )


def _blockdiag(a, b):
    out = np.zeros((a.shape[0] + b.shape[0], a.shape[1] + b.shape[1]), np.float32)
    out[: a.shape[0], : a.shape[1]] = a
    out[a.shape[0] :, a.shape[1] :] = b
    return out


def kernel(**inputs):
    global last_exec_ns
    x = np.ascontiguousarray(np.asarray(inputs["x"], np.float32))
    W1 = _expand(inputs["wp1"], inputs["w1"])  # [9, 49]
    W2 = _expand(inputs["wp2"], inputs["w2"])  # [49, 49]
    W3 = _expand(inputs["wp3"], inputs["w3"])  # [49, 9]
    B1 = _expand(inputs["bp1"], inputs["b1"])  # [49]
    B2 = _expand(inputs["bp2"], inputs["b2"])  # [49]
    B3 = _expand(inputs["bp3"], inputs["b3"])  # [9]

    w1p = _blockdiag(W1, W1)                   # [18, 98]
    w1full = np.zeros((128, 98), np.float32)
    for j in range(4):
        w1full[32 * j : 32 * j + 18] = w1p
    w2p = np.ascontiguousarray(_blockdiag(W2, W2))   # [98, 98]
    w3p = _blockdiag(W3, W3)                   # [98, 18]
    w3x = np.zeros((98, 4, 72), np.float32)
    for j in range(4):
        w3x[:, j, 18 * j : 18 * j + 18] = w3p
    b1p = np.ascontiguousarray(np.concatenate([B1, B1])[:, None])  # [98, 1]
    b2p = np.ascontiguousarray(np.concatenate([B2, B2])[:, None])  # [98, 1]
    b3p = np.ascontiguousarray(np.tile(B3, 8)[:, None])            # [72, 1]

    xT = x.T  # [9, BATCH] view
    in_maps = []
    for c in range(N_CORES):
        xc = xT[:, c * BS : (c + 1) * BS]          # [9, BS]
        xt4 = np.ascontiguousarray(
            xc.reshape(9, NBLK, S).transpose(1, 0, 2).reshape(4, 18, S)
        )
        in_maps.append(
            {
                "xt": xt4,
                "w1": w1full,
                "w2": w2p,
                "w3x": w3x,
                "b1": b1p,
                "b2": b2p,
                "b3": b3p,
            }
        )

    use_f32 = os.environ.get("KERNEL_DTYPE", "f32r") == "f32"
    nc = _get_nc(f32 if use_f32 else f32r)
    trace = os.environ.get("KERNEL_TRACE", "0") == "1"
    # The axon-tunneled NRT intermittently fails with
    # NRT_EXEC_UNIT_UNRECOVERABLE; a plain retry recovers it.
    last_err = None
    for attempt in range(4):
        try:
            res = run_bass_kernel_spmd(
                nc, in_maps, core_ids=list(range(N_CORES)), trace=trace
            )
            break
        except Exception as e:  # noqa: BLE001
            last_err = e
            import time as _time

            _time.sleep(2.0 * (attempt + 1))
    else:
        raise last_err
    if trace:
        last_exec_ns = res.exec_time_ns

    y = np.empty((BATCH, 9), np.float32)
    for c in range(N_CORES):
        ytc = res.results[c]["yt"]  # [72, S]
        # row 18j + 9h + f  <->  block k=2j+h, feature f
        yTc = ytc.reshape(4, 2, 9, S).transpose(2, 0, 1, 3).reshape(9, BS)
        y[c * BS : (c + 1) * BS] = yTc.T
    return y



# revision 16
# speedup vs baseline: 1.5077x; 1.5077x over previous
"""Equivariant MLP (9 -> 49 -> 49 -> 9, tied weights) on 8 trn2 NeuronCores.

Data parallel over the batch (1048576 rows -> 131072/core).  Tied-weight
patterns are expanded to dense matrices on the host.  Per core the batch is
8 blocks of 16384 columns; band/pair j stacks blocks (2j, 2j+1) on 9+9
feature partitions, giving x^T as [72, 16384] in SBUF-ready layout.

Loop over 32 column-chunks of 512.  Per pair: L1 matmul (fp32r, stationary
[18,98] block-diagonal weights) into a 1-bank PSUM tile, relu+bias
evacuation; L2 matmul (stationary [98,99] block-diagonal weights whose 99th
column is zero) writes pair-couples into one [99,1024] 2-bank PSUM tile so
one wide evacuation (bias vector carries 1.0 at row 98 -> relu gives a
constant ones row) produces fp16 h2 with a built-in bias row.  L3 flips
orientation: h2 [99,128] chunks are the *stationary* operand and the tiny
fp16 [99,18] weight block (W3 blocks + b3 row) is the moving operand, so
each matmul streams only 18 output columns and lands y batch-major in
PSUM, which is DMA'd straight to HBM -- no L3 evacuation instruction and
b3 comes along via the ones row.  Evacuations rotate across the ACT, DVE
and GpSimd engines so no single vector engine exceeds the PE's ~1.8us per
chunk.
"""

import os
import sys

sys.path.insert(0, "/opt/trn_rl_repo")

import numpy as np

import concourse.bass as bass
import concourse.mybir as mybir
import concourse.tile as tile
from concourse.bass_utils import run_bass_kernel_spmd

f32 = mybir.dt.float32
f32r = mybir.dt.float32r
f16 = mybir.dt.float16

N_CORES = 8
BATCH = 1048576
BS = BATCH // N_CORES          # 131072 rows per core
S = BS // 8                    # 16384 columns per block
C = 512                        # columns per chunk
NCH = S // C                   # 32 chunks

last_exec_ns = None


def _split_multi_waits(nc):
    """Walrus in this container rejects instructions carrying more than one
    sync wait ("Too many sync wait commands", e.g. Drain and Ldweights
    encodings).  Re-park all but one wait of every instruction on same-engine
    NoOps inserted just before it."""
    n = 0
    for fn in nc.m.functions:
        for bb in fn.blocks:
            out = []
            for inst in bb.instructions:
                si = inst.sync_info
                waits = list(si.on_wait) if (si and si.on_wait) else []
                if len(waits) > 1:
                    si.on_wait = waits[-1:]
                    for w in waits[:-1]:
                        nop = mybir.InstNoOp(name=f"WSPLIT-{n}", ins=[], outs=[])
                        n += 1
                        nop.engine = inst.engine
                        nop.sync_info = mybir.SyncInfo(on_update=[], on_wait=[w])
                        out.append(nop)
                out.append(inst)
            bb.instructions = out


def _build_nc():
    nc = bass.Bass()
    xt = nc.dram_tensor("xt", [128, S], f32r, kind="ExternalInput")
    w1 = nc.dram_tensor("w1", [128, 98], f32r, kind="ExternalInput")
    w2 = nc.dram_tensor("w2", [98, 99], f32r, kind="ExternalInput")
    w3 = nc.dram_tensor("w3", [99, 18], f16, kind="ExternalInput")
    b1 = nc.dram_tensor("b1", [98, 1], f32, kind="ExternalInput")
    b2 = nc.dram_tensor("b2", [99, 1], f32, kind="ExternalInput")
    yt = nc.dram_tensor("yt", [NCH, 128, 2, 2, 4, 18], f32, kind="ExternalOutput")

    relu = mybir.ActivationFunctionType.Relu
    add = mybir.AluOpType.add
    amax = mybir.AluOpType.max

    with tile.TileContext(nc) as tc:
        with (
            tc.tile_pool(name="consts", bufs=1) as cp,
            tc.tile_pool(name="xs", bufs=3) as xp,
            tc.tile_pool(name="hid1", bufs=6) as hp,
            tc.tile_pool(name="hid2", bufs=6) as h2p,
            tc.tile_pool(name="ys", bufs=3) as ypl,
            tc.tile_pool(name="psA", bufs=3, space=bass.MemorySpace.PSUM) as ppa,
            tc.tile_pool(name="psB", bufs=2, space=bass.MemorySpace.PSUM) as ppb,
            tc.tile_pool(name="psC", bufs=1, space=bass.MemorySpace.PSUM) as ppc,
        ):
            # x(0)/x(1) prefetches go first so the five constant DMAs do
            # not serialize ahead of them on the shared HWDGE device.
            w1t = cp.tile([128, 98], f32r)
            w2t = cp.tile([98, 99], f32r)
            w3t = cp.tile([99, 18], f16)
            b1t = cp.tile([98, 1], f32)
            b2t = cp.tile([99, 1], f32)

            # Software-pipelined across iterations with a 2-deep skew so every
            # cross-engine dependency has about a full iteration of slack:
            # iteration i runs L1(i) on PE interleaved with L3(i-2) and
            # L2(i-1); evacuations rotate ACT/DVE/Pool/Pool (L1), ACT/DVE
            # (L2-couples) and the p3->SBUF y copy splits ACT/Pool.
            def evac1(j, dst, src):
                if j % 2 == 0:
                    nc.scalar.activation(dst, src, relu, bias=b1t[:, 0:1])
                else:
                    nc.vector.tensor_scalar(
                        dst, src, b1t[:, 0:1], 0.0, add, amax
                    )

            def mm1(xtl, j):
                p1 = ppa.tile([98, C], f32, tag="p1", name=f"p1_{j}")
                kw = {"tile_position": (96, 0)} if j == 3 else {}
                nc.tensor.matmul(
                    p1[:],
                    w1t[32 * j : 32 * j + 18, :],
                    xtl[32 * j : 32 * j + 18, :],
                    start=True,
                    stop=True,
                    **kw,
                )
                h1 = hp.tile([98, C], f32r, tag="h1", name=f"h1_{j}")
                evac1(j, h1[:], p1[:])
                return h1

            def mm3(h2s, p3, jj):
                h2 = h2s[jj]
                for h in range(2):
                    for k in range(4):
                        q = (2 * jj + h) * 4 + k
                        nc.tensor.matmul(
                            p3[:, q, :],
                            h2[:, (4 * h + k) * 128 : (4 * h + k + 1) * 128],
                            w3t[:],
                            start=True,
                            stop=True,
                        )

            def emit_y(p3, ci):
                ytl = ypl.tile([128, 16, 18], f32, tag="y")
                nc.scalar.copy(ytl[:], p3[:])
                nc.sync.dma_start(yt[ci], ytl[:])

            def load_x(ci):
                xtl = xp.tile([128, C], f32r, tag="x", name=f"x_{ci}")
                nc.sync.dma_start(xtl[:], xt[:, ci * C : ci * C + C])
                return xtl

            def mm2(h1s, jj):
                p2 = ppb.tile([99, 2 * C], f32, tag="p2", name=f"p2_{jj}")
                for h in range(2):
                    nc.tensor.matmul(
                        p2[:, h * C : (h + 1) * C],
                        w2t[:],
                        h1s[2 * jj + h][:],
                        start=True,
                        stop=True,
                    )
                h2 = h2p.tile([99, 2 * C], f16, tag="h2", name=f"h2_{jj}")
                if jj == 0:
                    nc.vector.tensor_scalar(
                        h2[:], p2[:], b2t[:, 0:1], 0.0, add, amax
                    )
                else:
                    nc.scalar.activation(h2[:], p2[:], relu, bias=b2t[:, 0:1])
                return h2

            xq = [load_x(0), load_x(1)]   # prefetched x tiles, head = next
            nc.sync.dma_start(w1t[:], w1[:])
            nc.sync.dma_start(b1t[:], b1[:])
            nc.sync.dma_start(w2t[:], w2[:])
            nc.sync.dma_start(b2t[:], b2[:])
            nc.sync.dma_start(w3t[:], w3[:])
            h1_prev = None                # h1 tiles of iter i-1
            h2_prev = None                # h2 tiles of iter i-2
            h2_prev2 = None               # h2 tiles of iter i-3 (L3 input)
            for ci in range(NCH + 3):
                if ci < NCH:
                    xtl = xq.pop(0)
                    if ci + 2 < NCH:
                        xq.append(load_x(ci + 2))
                    h1s = [mm1(xtl, 0), mm1(xtl, 1), mm1(xtl, 2)]
                else:
                    h1s = None
                if h2_prev2 is not None:
                    p3 = ppc.tile([128, 16, 18], f32, tag="p3")
                    mm3(h2_prev2, p3, 0)
                    mm3(h2_prev2, p3, 1)
                    emit_y(p3, ci - 3)
                h2_cur = None
                if h1_prev is not None:
                    h2_cur = [mm2(h1_prev, 0)]
                if h1s is not None:
                    h1s.append(mm1(xtl, 3))
                if h1_prev is not None:
                    h2_cur.append(mm2(h1_prev, 1))
                h1_prev = h1s
                h2_prev2 = h2_prev
                h2_prev = h2_cur
    _split_multi_waits(nc)
    return nc


_nc_cache = {}


def _get_nc(mm_dtype=None):
    key = "v2"
    if key not in _nc_cache:
        _nc_cache[key] = _build_nc()
    return _nc_cache[key]


def _expand(pattern, params):
    pattern = np.asarray(pattern)
    params = np.asarray(params, np.float32)
    return np.where(pattern > 0, params[np.maximum(pattern - 1, 0)], 0.0).astype(
        np.float32
    )


def _blockdiag(a, b):
    out = np.zeros((a.shape[0] + b.shape[0], a.shape[1] + b.shape[1]), np.float32)
    out[: a.shape[0], : a.shape[1]] = a
    out[a.shape[0] :, a.shape[1] :] = b
    return out


def kernel(**inputs):
    global last_exec_ns
    x = np.ascontiguousarray(np.asarray(inputs["x"], np.float32))
    W1 = _expand(inputs["wp1"], inputs["w1"])  # [9, 49]
    W2 = _expand(inputs["wp2"], inputs["w2"])  # [49, 49]
    W3 = _expand(inputs["wp3"], inputs["w3"])  # [49, 9]
    B1 = _expand(inputs["bp1"], inputs["b1"])  # [49]
    B2 = _expand(inputs["bp2"], inputs["b2"])  # [49]
    B3 = _expand(inputs["bp3"], inputs["b3"])  # [9]

    w1b = _blockdiag(W1, W1)                                 # [18, 98]
    w1p = np.zeros((128, 98), np.float32)
    for j in range(4):
        w1p[32 * j : 32 * j + 18] = w1b
    w2p = np.zeros((98, 99), np.float32)
    w2p[:, :98] = _blockdiag(W2, W2)                         # 99th col zero
    w3p = np.zeros((99, 18), np.float32)
    w3p[:98, :] = _blockdiag(W3, W3)
    w3p[98, :9] = B3
    w3p[98, 9:] = B3
    w3p = w3p.astype(np.float16)
    b1p = np.ascontiguousarray(np.concatenate([B1, B1])[:, None])  # [98, 1]
    b2p = np.concatenate([B2, B2, [1.0]]).astype(np.float32)[:, None]  # [99, 1]

    xT = x.T  # [9, BATCH] view
    in_maps = []
    for c in range(N_CORES):
        xc = xT[:, c * BS : (c + 1) * BS]          # [9, BS]
        xb = xc.reshape(9, 8, S).transpose(1, 0, 2).reshape(4, 18, S)
        xt4 = np.zeros((128, S), np.float32)
        for j in range(4):
            xt4[32 * j : 32 * j + 18] = xb[j]
        in_maps.append(
            {
                "xt": xt4,
                "w1": w1p,
                "w2": w2p,
                "w3": w3p,
                "b1": b1p,
                "b2": b2p,
            }
        )

    nc = _get_nc()
    trace = os.environ.get("KERNEL_TRACE", "0") == "1"
    # The axon-tunneled NRT intermittently fails with
    # NRT_EXEC_UNIT_UNRECOVERABLE; a plain retry recovers it.
    last_err = None
    for attempt in range(4):
        try:
            res = run_bass_kernel_spmd(
                nc, in_maps, core_ids=list(range(N_CORES)), trace=trace
            )
            break
        except Exception as e:  # noqa: BLE001
            last_err = e
            import time as _time

            _time.sleep(2.0 * (attempt + 1))
    else:
        raise last_err
    if trace:
        last_exec_ns = res.exec_time_ns

    y = np.empty((BATCH, 9), np.float32)
    for c in range(N_CORES):
        ytc = res.results[c]["yt"]  # [NCH, 128, 2, 2, 4, 18]
        # element (ci, m, jj, h, k, 9hh+f) -> block 2*(2jj+h)+hh? no:
        # pair j=2jj+h covers blocks 2j (cols 0:9) and 2j+1 (cols 9:18);
        # within-block column t = 512*ci + 128*k + m.
        yc = ytc.reshape(NCH, 128, 4, 4, 2, 9)   # (ci, m, j, k, hh, f)
        # block b=2j+hh, row t=512*ci+128*k+m
        yc = yc.transpose(2, 4, 0, 3, 1, 5)      # (j, hh, ci, k, m, f)
        y[c * BS : (c + 1) * BS] = yc.reshape(BS, 9)
    return y
